# revision 1
# baseline (speedup 1.0000x reference)
import numpy as np
import jax
import jax.numpy as jnp
from jax import lax
from jax.sharding import Mesh, NamedSharding, PartitionSpec as P

B, S, D, F = 2, 4096, 1024, 4096
LN_EPS = 1e-6


def _ln(x, scale, bias):
    mu = jnp.mean(x, axis=-1, keepdims=True)
    var = jnp.mean(jnp.square(x - mu), axis=-1, keepdims=True)
    return (x - mu) * lax.rsqrt(var + LN_EPS) * scale + bias


def _math(x, Wq, Wk, Wv, War, Wai, Wg, Wo, ln1s, ln1b, W1, b1, W2, b2, ln2s, ln2b):
    q = x @ Wq
    k = x @ Wk
    v = x @ Wv
    ar_raw = x @ War
    ai_raw = x @ Wai
    # a_c = sigmoid(|a|) * exp(i*angle(a)) = sigmoid(mag)/mag * (ar + i*ai)
    mag = jnp.sqrt(ar_raw * ar_raw + ai_raw * ai_raw)
    sg = jax.nn.sigmoid(mag) / mag
    a_r = sg * ar_raw
    a_i = sg * ai_raw
    kv = k * v

    # complex linear recurrence h_t = a_t*h_{t-1} + kv_t in real arithmetic.
    # Two-level scan: Hillis-Steele within chunks of L, tiny cross-chunk scan,
    # then one apply pass. Identity element is a=1+0i, x=0.
    L = 16
    NC = S // L
    b = x.shape[0]
    C = a_r.shape[-1]
    ar = a_r.reshape(b, NC, L, C)
    ai = a_i.reshape(b, NC, L, C)
    xr = kv.reshape(b, NC, L, C)
    xi = jnp.zeros_like(xr)
    d = 1
    while d < L:
        one = jnp.ones_like(ar[:, :, :d])
        zro = jnp.zeros_like(ar[:, :, :d])
        ar1 = jnp.concatenate([one, ar[:, :, :-d]], axis=2)
        ai1 = jnp.concatenate([zro, ai[:, :, :-d]], axis=2)
        xr1 = jnp.concatenate([zro, xr[:, :, :-d]], axis=2)
        xi1 = jnp.concatenate([zro, xi[:, :, :-d]], axis=2)
        nar = ar1 * ar - ai1 * ai
        nai = ar1 * ai + ai1 * ar
        nxr = ar * xr1 - ai * xi1 + xr
        nxi = ar * xi1 + ai * xr1 + xi
        ar, ai, xr, xi = nar, nai, nxr, nxi
        d *= 2
    # inclusive scan over per-chunk summaries (small arrays)
    car = ar[:, :, -1]
    cai = ai[:, :, -1]
    cxr = xr[:, :, -1]
    cxi = xi[:, :, -1]
    d = 1
    while d < NC:
        one = jnp.ones_like(car[:, :d])
        zro = jnp.zeros_like(car[:, :d])
        ar1 = jnp.concatenate([one, car[:, :-d]], axis=1)
        ai1 = jnp.concatenate([zro, cai[:, :-d]], axis=1)
        xr1 = jnp.concatenate([zro, cxr[:, :-d]], axis=1)
        xi1 = jnp.concatenate([zro, cxi[:, :-d]], axis=1)
        nar = ar1 * car - ai1 * cai
        nai = ar1 * cai + ai1 * car
        nxr = car * xr1 - cai * xi1 + cxr
        nxi = car * xi1 + cai * xr1 + cxi
        car, cai, cxr, cxi = nar, nai, nxr, nxi
        d *= 2
    # exclusive carry entering each chunk, applied in one pass (real part only)
    zc = jnp.zeros_like(cxr[:, :1])
    Hr = jnp.concatenate([zc, cxr[:, :-1]], axis=1)[:, :, None, :]
    Hi = jnp.concatenate([zc, cxi[:, :-1]], axis=1)[:, :, None, :]
    hr = (ar * Hr - ai * Hi + xr).reshape(b, S, C)

    # y = q*h ; y *= silu(g) (g real) ; only real part survives through @Wo
    g = x @ Wg
    y2 = q * hr * (g * jax.nn.sigmoid(g))
    attn = y2 @ Wo
    y = _ln(attn + x, ln1s, ln1b)
    h = jax.nn.gelu(y @ W1 + b1)
    ffn = h @ W2 + b2
    return _ln(ffn + y, ln2s, ln2b)


_CACHE = {}


def _get_fn():
    if "fn" in _CACHE:
        return _CACHE["fn"]
    devs = jax.devices()
    try:
        n = 8 if len(devs) >= 8 else len(devs)
        mesh = Mesh(np.array(devs[:n]), ("tp",))
        col = NamedSharding(mesh, P(None, "tp"))   # shard output channels
        row = NamedSharding(mesh, P("tp", None))   # shard input channels
        rep = NamedSharding(mesh, P())
        b1s = NamedSharding(mesh, P("tp"))
        in_sh = (rep, col, col, col, col, col, col, row,
                 rep, rep, col, b1s, row, rep, rep, rep)
        fn = jax.jit(_math, in_shardings=in_sh, out_shardings=rep)
        _CACHE["fn"] = (fn, in_sh)
    except Exception:
        fn = jax.jit(_math)
        _CACHE["fn"] = (fn, None)
    return _CACHE["fn"]


def kernel(**inputs):
    x = np.asarray(inputs["x"], np.float32)
    Wa = np.asarray(inputs["Wa"], np.float32)
    args = [
        x,
        np.asarray(inputs["Wq"], np.float32),
        np.asarray(inputs["Wk"], np.float32),
        np.asarray(inputs["Wv"], np.float32),
        np.ascontiguousarray(Wa[:, :D]),
        np.ascontiguousarray(Wa[:, D:]),
        np.asarray(inputs["Wg"], np.float32),
        np.asarray(inputs["Wo"], np.float32),
        np.asarray(inputs["ln1_scale"], np.float32),
        np.asarray(inputs["ln1_bias"], np.float32),
        np.asarray(inputs["W1"], np.float32),
        np.asarray(inputs["b1"], np.float32),
        np.asarray(inputs["W2"], np.float32),
        np.asarray(inputs["b2"], np.float32),
        np.asarray(inputs["ln2_scale"], np.float32),
        np.asarray(inputs["ln2_bias"], np.float32),
    ]
    fn, in_sh = _get_fn()
    try:
        if in_sh is not None:
            args = [jax.device_put(a, s) for a, s in zip(args, in_sh)]
        out = fn(*args)
        return np.asarray(out, np.float32)
    except Exception:
        fn1 = jax.jit(_math)
        _CACHE["fn"] = (fn1, None)
        out = fn1(*args)
        return np.asarray(out, np.float32)

